# revision 14
# baseline (speedup 1.0000x reference)
"""Trainium2 Bass kernel for nn_LSTMEncoder: 5-layer bidirectional LSTM (B=16,L=64,H=400)
+ pairwise quintic-poly MLP head, algebraically collapsed.

Sharding: 8 cores = 2 directions x 4 batch-groups (B=4/core). Direction is encoded in
per-core DATA (weights/masks/index order), program is identical (SPMD).
Per-layer dir-pair exchange via masked AllReduce (branch-free slot selection).

All inputs are packed into ONE [128, NBLOB] f16 tensor per core and the output is a
single [128, 64] f32 tensor: the axon/PJRT dispatch path costs ~1.5-2 ms per argument
per exec, so argument count dominates wall-clock; 1 input + 1 output runs at the
dispatch floor.
"""
import numpy as np
from contextlib import ExitStack

import concourse.bass as bass
import concourse.bacc as bacc
import concourse.tile as tile
from concourse import mybir
from concourse.bass_utils import run_bass_kernel_spmd

F32 = mybir.dt.float32
F16 = mybir.dt.float16
AF = mybir.ActivationFunctionType
ALU = mybir.AluOpType

H = 400
L = 64          # seq len / steps
B = 16          # total batch
BC = 4          # batch per core
NL = 5
NCORES = 8
GATE_SRC = [0, 1, 3, 2]   # q order (i,f,o,g) -> original gate block (i,f,g,o)

# ---------------- packed blob column layout (f16) ----------------
OFF_X0T = 0                      # 1024
OFF_WHH = 1024                   # 5 * 6400
OFF_WIH0 = OFF_WHH + 5 * 6400    # 6400
OFF_WIHR = OFF_WIH0 + 6400       # 4 * 12800
OFF_BIAS = OFF_WIHR + 4 * 12800  # 80
OFF_MASKS = OFF_BIAS + 80        # 4
OFF_CONSTS = OFF_MASKS + 4       # 8
OFF_W1 = OFF_CONSTS + 8          # 4 * 800 (ao, ap, bo, bp)
NBLOB = OFF_W1 + 4 * 800

# ---------------- M-tile geometry ----------------
# 16 M-tiles: m<12 -> (q=m//3, k=m%3), 128 rows; m>=12 -> q=m-12, k=3, 16 rows.
def mtile_info(m):
    if m < 12:
        q, k = divmod(m, 3)
        return q * 16 + k * 4, 128, (q * 3 + k) * 128, q, k
    q = m - 12
    return q * 16 + 12, 16, 1536 + q * 16, q, 3


def _col_order():
    """order[j] = original Whh row index placed at lhsT free-col j."""
    order = []
    for q in range(4):
        for k in range(3):
            for r in range(128):
                order.append(GATE_SRC[q] * 400 + k * 128 + r)
    for q in range(4):
        for r in range(16):
            order.append(GATE_SRC[q] * 400 + 384 + r)
    return np.array(order)

COL_ORDER = _col_order()


def _prep_lhsT(W, nhalves):
    """W: (1600, D) with D = 400*nhalves. Returns (4*nhalves, 128, 1600) fp16 lhsT tiles.
    Rows (contraction) are split into nhalves halves of 400, each zero-padded to 512."""
    Wr = W[COL_ORDER, :]                       # (1600, D) reordered gate rows
    halves = []
    for s in range(nhalves):
        h = Wr[:, s * 400:(s + 1) * 400]       # (1600, 400)
        h = np.concatenate([h, np.zeros((1600, 112), h.dtype)], axis=1)  # pad to 512
        halves.append(h)
    Wp = np.concatenate(halves, axis=1)        # (1600, 512*nh)
    lhsT = Wp.T.reshape(4 * nhalves, 128, 1600)
    return np.ascontiguousarray(lhsT.astype(np.float16))


def _prep_bias(bvec):
    """(1600,) -> (16,128) per-M-tile per-partition bias."""
    b = bvec[COL_ORDER]
    out = np.zeros((16, 128), np.float32)
    for m in range(16):
        pc, rows, wc, q, k = mtile_info(m)
        out[m, :rows] = b[wc:wc + rows]
    return out


def _prep_w1(W1h):
    """W1h: (100, 800) -> (8,128,100) lhsT tiles (two 400-halves padded to 512)."""
    T = W1h.T  # (800, 100)
    halves = [np.concatenate([T[s * 400:(s + 1) * 400], np.zeros((112, 100), T.dtype)], 0)
              for s in range(2)]
    return np.ascontiguousarray(np.concatenate(halves, 0).reshape(8, 128, 100))


def _kmaj(a):
    """(k, 128, n) -> (128, k*n) flat k-major per partition."""
    return np.ascontiguousarray(a.transpose(1, 0, 2).reshape(128, -1))


# ---------------- device program ----------------
def build_program(no_collective=False, n_layers=NL, no_head=False):
    nc = bacc.Bacc("TRN2", target_bir_lowering=False, debug=False, num_devices=NCORES)
    dp = nc.declare_dram_parameter
    blob_d = dp("blob", [128, NBLOB], F16, isOutput=False)
    out_d = dp("out", [128, 64], F32, isOutput=True)

    groups = [[g, g + 4] for g in range(4)]

    with tile.TileContext(nc) as tc, ExitStack() as ctx:
        pool1 = ctx.enter_context(tc.tile_pool(name="persist", bufs=1))
        whhp = ctx.enter_context(tc.tile_pool(name="whh", bufs=2))
        wihp = ctx.enter_context(tc.tile_pool(name="wih", bufs=2))
        xsp = ctx.enter_context(tc.tile_pool(name="xs", bufs=1))
        xop = ctx.enter_context(tc.tile_pool(name="xout", bufs=2))
        xip = ctx.enter_context(tc.tile_pool(name="xin", bufs=2))
        gp = ctx.enter_context(tc.tile_pool(name="gates", bufs=3))
        sp = ctx.enter_context(tc.tile_pool(name="small", bufs=4))
        php = ctx.enter_context(tc.tile_pool(name="phase", bufs=1))
        gps = ctx.enter_context(tc.tile_pool(name="gpsum", bufs=2, space="PSUM"))
        xps = ctx.enter_context(tc.tile_pool(name="xpsum", bufs=3, space="PSUM"))
        pps = ctx.enter_context(tc.tile_pool(name="ppsum", bufs=2, space="PSUM"))
        drp = ctx.enter_context(tc.tile_pool(name="dram", bufs=2, space="DRAM"))

        # ---- persistent loads (all contiguous slices of the blob) ----
        x0T = pool1.tile([128, 16 * L], F16, tag="x0T")
        nc.sync.dma_start(x0T[:], blob_d[:, OFF_X0T:OFF_X0T + 1024])
        # scalar-pointer operands must be f32: load f16 slices, upcast once
        scal16 = pool1.tile([128, 80 + 4 + 8], F16, tag="scal16")
        nc.sync.dma_start(scal16[:], blob_d[:, OFF_BIAS:OFF_BIAS + 92])
        bias_sb = pool1.tile([128, NL * 16], F32, tag="bias")
        nc.vector.tensor_copy(bias_sb[:], scal16[:, 0:80])
        masks_sb = pool1.tile([128, 4], F32, tag="masks")
        nc.vector.tensor_copy(masks_sb[:], scal16[:, 80:84])
        consts_sb = pool1.tile([128, 8], F32, tag="consts")
        nc.vector.tensor_copy(consts_sb[:], scal16[:, 84:92])
        consts16_sb = scal16
        w1_sb = {}
        for i, nm in enumerate(("ao", "ap", "bo", "bp")):
            t = pool1.tile([128, 800], F16, tag="w1" + nm)
            nc.sync.dma_start(t[:], blob_d[:, OFF_W1 + i * 800:OFF_W1 + (i + 1) * 800])
            w1_sb[nm] = t

        # persistent xs buffers (memset once; garbage cols stay 0)
        xs_own = pool1.tile([128, L * 64], F32, tag="xs_own")
        xs_part = pool1.tile([128, L * 64], F32, tag="xs_part")
        nc.vector.memset(xs_own[:], 0.0)
        nc.vector.memset(xs_part[:], 0.0)
        # zero both gate-psum slots once (garbage lanes read later must be finite 0)
        for _ in range(2):
            z = gps.tile([128, 64], F32, tag="gps")
            nc.vector.memset(z[:], 0.0)

        def xs_matmul(lhsT_sb, ktiles, rhs_tiles, lay, into_own_only):
            """xs_{own,part}[t*64+pc] += sum_k lhsT[k].T @ rhs[k]; evac with bias."""
            xov = xs_own[:].rearrange("p (t c) -> p t c", c=64)
            xpv = xs_part[:].rearrange("p (t c) -> p t c", c=64)
            for m in range(16):
                pc, rows, wc, q, k0 = mtile_info(m)
                pso = xps.tile([128, 256], F32, tag="xps")
                psp = None if into_own_only else xps.tile([128, 256], F32, tag="xps")
                for k in range(ktiles):
                    lw = lhsT_sb[:, k * 1600 + wc: k * 1600 + wc + rows]
                    ro, rp = rhs_tiles(k)
                    nc.tensor.matmul(pso[:rows, :], lw, ro, start=(k == 0), stop=(k == ktiles - 1))
                    if psp is not None:
                        nc.tensor.matmul(psp[:rows, :], lw, rp, start=(k == 0), stop=(k == ktiles - 1))
                biasap = bias_sb[:rows, lay * 16 + m: lay * 16 + m + 1]
                pv = pso[:rows, :].rearrange("p (t b) -> p t b", b=4)
                nc.vector.tensor_scalar(xov[:rows, :, pc:pc + 4], pv, biasap, None,
                                        op0=ALU.add)
                if psp is not None:
                    pv2 = psp[:rows, :].rearrange("p (t b) -> p t b", b=4)
                    nc.vector.tensor_copy(xpv[:rows, :, pc:pc + 4], pv2)

        # ---- layer 0 xs ----
        x0v = x0T[:].rearrange("p (t c) -> p t c", c=16)
        wih0_sb = wihp.tile([128, 4 * 1600], F16, tag="wih", bufs=1)
        nc.sync.dma_start(wih0_sb[:], blob_d[:, OFF_WIH0:OFF_WIH0 + 6400])
        xs_matmul(wih0_sb, 4, lambda k: (x0v[:, :, k * 4:(k + 1) * 4], None), 0, True)

        def load_whh(l):
            t = whhp.tile([128, 4 * 1600], F16, tag="whh", name=f"whh{l}")
            nc.sync.dma_start(t[:], blob_d[:, OFF_WHH + l * 6400:OFF_WHH + (l + 1) * 6400])
            return t

        def load_wihr(l):
            t = wihp.tile([128, 8 * 1600], F16, tag="wihr", name=f"wihr{l}")
            nc.sync.dma_start(t[:], blob_d[:, OFF_WIHR + (l - 1) * 12800:OFF_WIHR + l * 12800])
            return t

        whh_l = load_whh(0)
        wihr_next = load_wihr(1) if n_layers > 1 else None
        xin_f = None  # final exchange tile
        for l in range(n_layers):
            two = l > 0  # whether xs_part participates
            if l > 0:
                whh_l = load_whh(l)
            xout = xop.tile([128, 16 * L], F16, tag="xout")
            xov_ = xout[:].rearrange("p (t c) -> p t c", c=16)
            c_t = sp.tile([128, 16], F32, tag="c")
            nc.vector.memset(c_t[:], 0.0)
            for t in range(L):
                g = gp.tile([128, 64], F32, tag="g")
                if t == 0:
                    src = xs_own[:, 0:64]
                    nc.scalar.activation(g[:, 0:48], src[:, 0:48], AF.Sigmoid)
                    nc.scalar.activation(g[:, 48:64], src[:, 48:64], AF.Tanh)
                else:
                    ps = gps.tile([128, 64], F32, tag="gps")
                    for m in range(16):
                        pc, rows, wc, q, k0 = mtile_info(m)
                        for k in range(4):
                            nc.tensor.matmul(
                                ps[:rows, pc:pc + 4],
                                whh_l[:, k * 1600 + wc: k * 1600 + wc + rows],
                                xov_[:, t - 1, k * 4:(k + 1) * 4],
                                start=(k == 0), stop=(k == 3))
                    nc.vector.tensor_add(g[:], ps[:], xs_own[:, t * 64:(t + 1) * 64])
                    nc.scalar.activation(g[:, 0:48], g[:, 0:48], AF.Sigmoid)
                    nc.scalar.activation(g[:, 48:64], g[:, 48:64], AF.Tanh)
                tmp = sp.tile([128, 16], F32, tag="tmp")
                nc.vector.tensor_mul(tmp[:], g[:, 0:16], g[:, 48:64])      # i*tanh(g)
                nc.vector.tensor_mul(c_t[:], g[:, 16:32], c_t[:])          # f*c
                nc.vector.tensor_add(c_t[:], c_t[:], tmp[:])
                thc = sp.tile([128, 16], F32, tag="thc")
                nc.scalar.activation(thc[:], c_t[:], AF.Tanh)
                nc.vector.tensor_mul(xov_[:, t, :], g[:, 32:48], thc[:])   # h (fp16 out)

            # ---- exchange (masked AllReduce over dir pair) ----
            stg = xip.tile([128, 2 * 16 * L], F16, tag="stg")
            for s in range(2):
                nc.vector.tensor_scalar_mul(stg[:, s * 1024:(s + 1) * 1024], xout[:],
                                            masks_sb[:, s:s + 1])
            in_b = drp.tile([128, 2048], F16, tag="arin")
            out_b = drp.tile([128, 2048], F16, tag="arout")
            nc.sync.dma_start(in_b[:], stg[:])
            if no_collective:
                nc.sync.dma_start(out_b[:], in_b[:])
            else:
                nc.gpsimd.collective_compute(
                    "AllReduce", ALU.add, ins=[in_b[:].opt()], outs=[out_b[:].opt()],
                    replica_groups=groups)
            xin = xip.tile([128, 2 * 16 * L], F16, tag="xin")
            nc.sync.dma_start(xin[:], out_b[:])
            if l < n_layers - 1:
                xin_o = xip.tile([128, 2048], F16, tag="xin_o", bufs=1)
                xin_p = xip.tile([128, 2048], F16, tag="xin_p", bufs=1)
                for s in range(2):
                    sl = slice(s * 1024, (s + 1) * 1024)
                    nc.vector.tensor_scalar_mul(xin_o[:, sl], xin[:, sl], masks_sb[:, s:s + 1])
                    nc.vector.tensor_scalar_mul(xin_p[:, sl], xin[:, sl], masks_sb[:, 2 + s:3 + s])
                xiov = xin_o[:].rearrange("p (s t c) -> p s t c", s=2, c=16)
                xipv = xin_p[:].rearrange("p (s t c) -> p s t c", s=2, c=16)
                wihr_cur, wihr_next = wihr_next, (load_wihr(l + 2) if l + 2 < n_layers else None)
                xs_matmul(wihr_cur, 8,
                          lambda k: (xiov[:, k // 4, :, (k % 4) * 4:(k % 4) * 4 + 4],
                                     xipv[:, k // 4, :, (k % 4) * 4:(k % 4) * 4 + 4]),
                          l + 1, False)
                # fold xs_part (read time-reversed) into xs_own once per layer
                # so the recurrence needs a single add per step
                xoc = xs_own[:].rearrange("p (t c) -> p t c", c=64)
                xpc = xs_part[:].rearrange("p (t c) -> p t c", c=64)
                nc.vector.tensor_add(xoc, xoc, xpc[:, ::-1, :])
            else:
                xin_f = xin

        # ---------------- phases ----------------
        if no_head:
            out_sb0 = php.tile([128, 64], F32, tag="osb0")
            nc.vector.tensor_copy(out_sb0[:, 0:64], xin_f[:, 0:64])
            nc.sync.dma_start(out_d[:, :], out_sb0[:])
        else:
            # P = poly(x) per slot, then masked own/part
            # quintic via 5 fused ops: P = ((((w5 f + w4) f + w3) f + w2) f + w1) f
            P = php.tile([128, 2048], F32, tag="P")
            x_f = xin_f[:]
            nc.vector.tensor_scalar(P[:], x_f, consts_sb[:, 4:5], consts_sb[:, 3:4],
                                    op0=ALU.mult, op1=ALU.add)
            t1 = php.tile([128, 2048], F32, tag="t1")
            nc.vector.scalar_tensor_tensor(t1[:], P[:], 0.0, x_f,
                                           op0=ALU.add, op1=ALU.mult)
            nc.vector.scalar_tensor_tensor(P[:], t1[:], consts_sb[:, 2:3], x_f,
                                           op0=ALU.add, op1=ALU.mult)
            nc.vector.scalar_tensor_tensor(t1[:], P[:], consts_sb[:, 1:2], x_f,
                                           op0=ALU.add, op1=ALU.mult)
            nc.vector.scalar_tensor_tensor(P[:], t1[:], consts_sb[:, 0:1], x_f,
                                           op0=ALU.add, op1=ALU.mult)
            Po = php.tile([128, 2048], F16, tag="Po")
            Pp = php.tile([128, 2048], F16, tag="Pp")
            for s in range(2):
                sl = slice(s * 1024, (s + 1) * 1024)
                nc.vector.tensor_scalar_mul(Po[:, sl], P[:, sl], masks_sb[:, s:s + 1])
                nc.vector.tensor_scalar_mul(Pp[:, sl], P[:, sl], masks_sb[:, 2 + s:3 + s])
            Pov = Po[:].rearrange("p (s t c) -> p s t c", s=2, c=16)
            Ppv = Pp[:].rearrange("p (s t c) -> p s t c", s=2, c=16)

            def halfmat(wown, wpart):
                pso = pps.tile([128, 256], F32, tag="pps")
                psp = pps.tile([128, 256], F32, tag="pps")
                for k in range(8):
                    s, j = k // 4, k % 4
                    nc.tensor.matmul(pso[:100, :], wown[:, k * 100:(k + 1) * 100],
                                     Pov[:, s, :, j * 4:j * 4 + 4], start=(k == 0), stop=(k == 7))
                    nc.tensor.matmul(psp[:100, :], wpart[:, k * 100:(k + 1) * 100],
                                     Ppv[:, s, :, j * 4:j * 4 + 4], start=(k == 0), stop=(k == 7))
                return pso, psp

            def rev_add(dst, pso, psp):
                pv = psp[:100, :].rearrange("p (t b) -> p t b", b=4)
                dv = dst[:100, :].rearrange("p (t b) -> p t b", b=4)
                nc.vector.tensor_copy(dst[:100, :], pso[:100, :])
                nc.vector.tensor_add(dv, dv, pv[:, ::-1, :])

            A_sb = php.tile([128, 256], F32, tag="A")
            C_sb = php.tile([128, 256], F32, tag="C")
            pso, psp = halfmat(w1_sb["ao"], w1_sb["ap"])
            rev_add(A_sb, pso, psp)
            nc.vector.tensor_scalar(A_sb[:100, :], A_sb[:100, :], consts_sb[:100, 5:6], None,
                                    op0=ALU.add)  # + b1
            pso, psp = halfmat(w1_sb["bo"], w1_sb["bp"])
            rev_add(C_sb, pso, psp)

            out_sb = php.tile([128, 64], F32, tag="osb")
            Cv = C_sb[:100, :].rearrange("p (t b) -> p t b", b=4)
            ps4 = pps.tile([128, 64], F32, tag="ps4", bufs=1)
            for a in range(32):
                hm = gp.tile([128, 256], F16, tag="hm")
                for b in range(4):
                    nc.vector.tensor_scalar(hm[:100, b * 64:(b + 1) * 64], Cv[:, :, b],
                                            A_sb[:100, a * 4 + b:a * 4 + b + 1], 0.0,
                                            op0=ALU.add, op1=ALU.max)
                for ch in range(2):
                    # out[bc, 0] = sum_k hm[k, ch*128+bc] * W2[k]
                    nc.tensor.matmul(ps4[:, a * 2 + ch:a * 2 + ch + 1],
                                     hm[:100, ch * 128:(ch + 1) * 128],
                                     consts16_sb[:100, 84 + 7:84 + 8])
            nc.vector.tensor_scalar(out_sb[:, :], ps4[:, :], consts_sb[0:128, 6:7], None,
                                    op0=ALU.add)  # + b2
            nc.sync.dma_start(out_d[:, :], out_sb[:])
    nc.compile()
    return nc


_CACHE = {}


def _get_program():
    if "nc" not in _CACHE:
        _CACHE["nc"] = build_program()
    return _CACHE["nc"]


def _prep_core_inputs(c, words, pos, w_emb, t_emb, Wih0, Wih_rest, Whh, bih, bhh,
                      ws, mlp_W1, mlp_b1, mlp_W2, mlp_b2):
    d, g = c // 4, c % 4
    bs = slice(4 * g, 4 * g + 4)
    # x0T: (128, 1024) f16, col = t*16 + j*4 + b  (slot-order time)
    X = np.concatenate([w_emb[words[bs]], t_emb[pos[bs]]], axis=-1)  # (4,64,400)
    if d == 1:
        X = X[:, ::-1]
    Xp = np.concatenate([X, np.zeros((4, 64, 112), X.dtype)], -1)    # pad 512
    x0T = Xp.reshape(4, 64, 4, 128).transpose(3, 1, 2, 0).reshape(128, 1024)
    whhT = np.concatenate([_kmaj(_prep_lhsT(Whh[l, d], 1)) for l in range(NL)], axis=1)
    wih0T = _kmaj(_prep_lhsT(Wih0[d], 1))
    wihrT = np.concatenate([_kmaj(_prep_lhsT(Wih_rest[l - 1, d], 2))
                            for l in range(1, NL)], axis=1)
    bias = np.stack([_prep_bias(bih[l, d] + bhh[l, d]) for l in range(NL)])  # (5,16,128)
    bias = bias.reshape(NL * 16, 128).T                                      # (128, 80)
    masks = np.zeros((4, 128), np.float32)
    masks[0] = float(d == 0); masks[1] = float(d == 1)
    masks[2] = float(d == 1); masks[3] = float(d == 0)
    consts = np.zeros((8, 128), np.float32)
    for i in range(5):
        consts[i] = ws[i]
    consts[5, :100] = mlp_b1
    consts[6] = mlp_b2[0]
    consts[7, :100] = mlp_W2[0]
    w1a = _prep_w1(mlp_W1[:, :800])
    w1b = _prep_w1(mlp_W1[:, 800:])
    zero = np.zeros_like(w1a)
    sel = lambda W, own: np.concatenate(
        [(W[0:4] if (0 == d) == own else zero[0:4]),
         (W[4:8] if (1 == d) == own else zero[4:8])], 0)
    blob = np.concatenate([
        x0T.astype(np.float16),
        whhT, wih0T, wihrT,
        bias.astype(np.float16),
        masks.T.astype(np.float16),
        consts.T.astype(np.float16),
        _kmaj(sel(w1a, True)).astype(np.float16),
        _kmaj(sel(w1a, False)).astype(np.float16),
        _kmaj(sel(w1b, True)).astype(np.float16),
        _kmaj(sel(w1b, False)).astype(np.float16),
    ], axis=1)
    assert blob.shape == (128, NBLOB), blob.shape
    return {"blob": np.ascontiguousarray(blob)}


def _postprocess(results):
    out = np.zeros((L, B, L, 1), np.float32)
    ar = np.arange(32)
    ac = np.arange(L)
    for c in range(NCORES):
        d, g = c // 4, c % 4
        arr = results[c]["out"]                     # (128, 64): [p, a*2+ch]
        # value at [p, a*2+ch] belongs to (a, bl=ch*2+p//64, c=p%64)
        ch = arr.reshape(2, 64, 32, 2).transpose(2, 3, 0, 1).reshape(32, 4, 64)
        for bl in range(4):
            if d == 0:
                out[ar[:, None], 4 * g + bl, ac[None, :], 0] = ch[:, bl, :]
            else:
                out[(63 - ar)[:, None], 4 * g + bl, (63 - ac)[None, :], 0] = ch[:, bl, :]
    return out


def prepare(words_idx_tensor, pos_idx_tensor, max_length, w_emb, t_emb, Wih0, Wih_rest,
            Whh, bih, bhh, w1, w2, w3, w4, w5, mlp_W1, mlp_b1, mlp_W2, mlp_b2):
    words = np.asarray(words_idx_tensor)[:, :int(max_length)].astype(np.int64)
    pos = np.asarray(pos_idx_tensor)[:, :int(max_length)].astype(np.int64)
    assert words.shape == (B, L)
    args = tuple(np.asarray(x, np.float32) for x in
                 (w_emb, t_emb, Wih0, Wih_rest, Whh, bih, bhh))
    ws = [float(np.asarray(w).reshape(-1)[0]) for w in (w1, w2, w3, w4, w5)]
    mW1, mb1, mW2, mb2 = (np.asarray(mlp_W1, np.float32), np.asarray(mlp_b1, np.float32),
                          np.asarray(mlp_W2, np.float32), np.asarray(mlp_b2, np.float32))
    in_maps = [_prep_core_inputs(c, words, pos, *args, ws, mW1, mb1, mW2, mb2)
               for c in range(NCORES)]
    nc = _get_program()
    return nc, in_maps, _postprocess


def kernel(words_idx_tensor, pos_idx_tensor, max_length, w_emb, t_emb, Wih0, Wih_rest,
           Whh, bih, bhh, w1, w2, w3, w4, w5, mlp_W1, mlp_b1, mlp_W2, mlp_b2,
           _stats=None, _trace=False):
    nc, in_maps, post = prepare(
        words_idx_tensor, pos_idx_tensor, max_length, w_emb, t_emb, Wih0, Wih_rest,
        Whh, bih, bhh, w1, w2, w3, w4, w5, mlp_W1, mlp_b1, mlp_W2, mlp_b2)
    res = run_bass_kernel_spmd(nc, in_maps, list(range(NCORES)), trace=_trace)
    if _stats is not None:
        _stats["exec_time_ns"] = res.exec_time_ns
        _stats["mean_exec_time_ns"] = res.mean_exec_time_ns
        _stats["profile_json"] = res.profile_json
    return _postprocess(res.results)


# revision 28
# speedup vs baseline: 1.8648x; 1.8648x over previous
"""Trainium2 Bass kernel for nn_LSTMEncoder: 5-layer bidirectional LSTM (B=16,L=64,H=400)
+ pairwise quintic-poly MLP head, algebraically collapsed.

Sharding: 8 cores = 2 directions x 4 batch-groups (B=4/core). Direction is encoded in
per-core DATA (weights/masks/index order), program is identical (SPMD).

Per-exec wall-clock through the axon/PJRT path is dominated by dispatch overhead that
scales with argument COUNT (~1.5 ms/arg) and input BYTES (~17 GB/s). So:
- ONE [128, NBLOB] f16 input per core, ONE [128, 64] f32 output.
- Weights travel as fp8e3 (e3m4) bits inside the blob (scale S, dequant folded into
  the existing PSUM-evacuation ops) and are consumed directly as fp8 matmul lhsT.
- Each core uploads only a QUARTER of its direction's weight image; a 4-way on-device
  AllGather (within each direction group) reassembles it (~4x fewer input bytes).
- The per-layer direction exchange is an AllGather over {fwd,bwd} pairs (half the wire
  of the previous masked AllReduce, and no masking ops).
"""
import numpy as np
from contextlib import ExitStack

import concourse.bass as bass
import concourse.bacc as bacc
import concourse.tile as tile
from concourse import mybir
from concourse.bass_utils import run_bass_kernel_spmd

F32 = mybir.dt.float32
F16 = mybir.dt.float16
F8 = mybir.dt.float8e3
NP8 = mybir.dt.np(F8)
AF = mybir.ActivationFunctionType
ALU = mybir.AluOpType

H = 400
L = 64          # seq len / steps
B = 16          # total batch
BC = 4          # batch per core
NL = 5
NCORES = 8
GATE_SRC = [0, 1, 3, 2]   # q order (i,f,o,g) -> original gate block (i,f,g,o)

# fp8e3 transfer format for the bulk weight image: halves upload bytes but the
# ~1.5% quantization noise pushed end-to-end rel err to 3e-2 (> the 2e-2 gate),
# so it's off; the image travels as f16. Flip W8 to re-test.
W8 = False
WS = 48.0 if W8 else 1.0   # fp8 weight scale; dequant 1/WS folded into evacuation
IMG_DT_SLOTS = 2 if W8 else 1   # image elems per f16 blob slot

# ---- weight image (per direction), flat col offsets in image elems ----
IMG_WHH = 0                      # 5 * 6400
IMG_WIH0 = 5 * 6400              # 6400
IMG_WIHR = IMG_WIH0 + 6400       # 4 * 12800
IMG_COLS = IMG_WIHR + 4 * 12800  # 89600
QCOLS = IMG_COLS // 4            # 22400 image elems per core quarter

# ---- packed blob layout (f16 slots) ----
OFF_X0T = 0                      # 1024
OFF_SCAL = 1024                  # 92 = bias 80 + masks 4 + consts 8
OFF_W1 = OFF_SCAL + 92           # 2 * 800 f16 (w1a, w1b full; masked on device)
OFF_WQ = OFF_W1 + 1600           # QCOLS // IMG_DT_SLOTS
NBLOB = OFF_WQ + QCOLS // IMG_DT_SLOTS


# ---------------- M-tile geometry ----------------
# 16 M-tiles: m<12 -> (q=m//3, k=m%3), 128 rows; m>=12 -> q=m-12, k=3, 16 rows.
def mtile_info(m):
    if m < 12:
        q, k = divmod(m, 3)
        return q * 16 + k * 4, 128, (q * 3 + k) * 128, q, k
    q = m - 12
    return q * 16 + 12, 16, 1536 + q * 16, q, 3


def _col_order():
    """order[j] = original Whh row index placed at lhsT free-col j."""
    order = []
    for q in range(4):
        for k in range(3):
            for r in range(128):
                order.append(GATE_SRC[q] * 400 + k * 128 + r)
    for q in range(4):
        for r in range(16):
            order.append(GATE_SRC[q] * 400 + 384 + r)
    return np.array(order)

COL_ORDER = _col_order()


def _prep_lhsT(W, nhalves):
    """W: (1600, D) with D = 400*nhalves. Returns (4*nhalves, 128, 1600) f32 lhsT tiles.
    Rows (contraction) are split into nhalves halves of 400, each zero-padded to 512."""
    Wr = W[COL_ORDER, :]                       # (1600, D) reordered gate rows
    halves = []
    for s in range(nhalves):
        h = Wr[:, s * 400:(s + 1) * 400]       # (1600, 400)
        h = np.concatenate([h, np.zeros((1600, 112), h.dtype)], axis=1)  # pad to 512
        halves.append(h)
    Wp = np.concatenate(halves, axis=1)        # (1600, 512*nh)
    return np.ascontiguousarray(Wp.T.reshape(4 * nhalves, 128, 1600))


def _prep_bias(bvec):
    """(1600,) -> (16,128) per-M-tile per-partition bias."""
    b = bvec[COL_ORDER]
    out = np.zeros((16, 128), np.float32)
    for m in range(16):
        pc, rows, wc, q, k = mtile_info(m)
        out[m, :rows] = b[wc:wc + rows]
    return out


def _prep_w1(W1h):
    """W1h: (100, 800) -> (8,128,100) f32 lhsT tiles (two 400-halves padded to 512)."""
    T = W1h.T  # (800, 100)
    halves = [np.concatenate([T[s * 400:(s + 1) * 400], np.zeros((112, 100), T.dtype)], 0)
              for s in range(2)]
    return np.ascontiguousarray(np.concatenate(halves, 0).reshape(8, 128, 100))


def _kmaj(a):
    """(k, 128, n) -> (128, k*n) flat k-major per partition."""
    return np.ascontiguousarray(a.transpose(1, 0, 2).reshape(128, -1))


def _img_pack(a):
    """f32 (128, n) image section -> f16-slot view for the blob."""
    if not W8:
        return a.astype(np.float16)
    q = np.clip(a * WS, -15.0, 15.0).astype(NP8)
    return np.ascontiguousarray(q).view(np.uint8).view(np.float16)


# ---------------- device program ----------------
def build_program(no_collective=False, n_layers=NL, no_head=False, dbg=None):
    nc = bacc.Bacc("TRN2", target_bir_lowering=False, debug=False, num_devices=NCORES)
    dp = nc.declare_dram_parameter
    blob_d = dp("blob", [128, NBLOB], F16, isOutput=False)
    out_d = dp("out", [128, 64], F32, isOutput=True)

    groups = [[g, g + 4] for g in range(4)]
    wgroups = [[0, 1, 2, 3], [4, 5, 6, 7]]
    IMG_DT = F8 if W8 else F16

    agin = nc.dram_tensor("agin", [128, QCOLS], IMG_DT, kind="Internal")
    agout = nc.dram_tensor("agout", [4, 128, QCOLS], IMG_DT, kind="Internal")
    exin = [nc.dram_tensor(f"exin{l}", [128, 1024], F16, kind="Internal")
            for l in range(n_layers)]
    exout = [nc.dram_tensor(f"exout{l}", [2, 128, 1024], F16, kind="Internal")
             for l in range(n_layers)]

    with tile.TileContext(nc) as tc, ExitStack() as ctx:
        pool1 = ctx.enter_context(tc.tile_pool(name="persist", bufs=1))
        whhp = ctx.enter_context(tc.tile_pool(name="whh", bufs=2))
        wihp = ctx.enter_context(tc.tile_pool(name="wih", bufs=2))
        xop = ctx.enter_context(tc.tile_pool(name="xout", bufs=2))
        xip = ctx.enter_context(tc.tile_pool(name="xin", bufs=2))
        gp = ctx.enter_context(tc.tile_pool(name="gates", bufs=3))
        sp = ctx.enter_context(tc.tile_pool(name="small", bufs=4))
        php = ctx.enter_context(tc.tile_pool(name="phase", bufs=1))
        gps = ctx.enter_context(tc.tile_pool(name="gpsum", bufs=2, space="PSUM"))
        xps = ctx.enter_context(tc.tile_pool(name="xpsum", bufs=3, space="PSUM"))
        pps = ctx.enter_context(tc.tile_pool(name="ppsum", bufs=2, space="PSUM"))

        # ---- weight AllGather: quarter image -> full per-direction image ----
        wq_ap = blob_d[:, OFF_WQ:OFF_WQ + QCOLS // IMG_DT_SLOTS]
        nc.sync.dma_start(agin.ap(), wq_ap.bitcast(F8) if W8 else wq_ap)
        if no_collective:
            for r in range(4):
                nc.sync.dma_start(agout.ap()[r], agin.ap())
        else:
            nc.gpsimd.collective_compute(
                "AllGather", ALU.bypass, ins=[agin.ap().opt()],
                outs=[agout.ap().opt()], replica_groups=wgroups)

        def load_img(t, lo, n):
            """DMA fp8 image cols [lo, lo+n) from gathered quarters into tile t."""
            done = 0
            while done < n:
                r, roff = divmod(lo + done, QCOLS)
                take = min(n - done, QCOLS - roff)
                nc.sync.dma_start(t[:, done:done + take],
                                  agout.ap()[r][:, roff:roff + take])
                done += take

        # ---- persistent loads (contiguous slices of the blob) ----
        x0T = pool1.tile([128, 16 * L], F16, tag="x0T")
        nc.sync.dma_start(x0T[:], blob_d[:, OFF_X0T:OFF_X0T + 1024])
        scal16 = pool1.tile([128, 92], F16, tag="scal16")
        nc.sync.dma_start(scal16[:], blob_d[:, OFF_SCAL:OFF_SCAL + 92])
        bias_sb = pool1.tile([128, NL * 16], F32, tag="bias")
        nc.vector.tensor_copy(bias_sb[:], scal16[:, 0:80])
        masks_sb = pool1.tile([128, 4], F32, tag="masks")
        nc.vector.tensor_copy(masks_sb[:], scal16[:, 80:84])
        consts_sb = pool1.tile([128, 8], F32, tag="consts")
        nc.vector.tensor_copy(consts_sb[:], scal16[:, 84:92])
        # w1a/w1b arrive full; own/part variants are built by masking on device
        w1full = pool1.tile([128, 1600], F16, tag="w1full")
        nc.sync.dma_start(w1full[:], blob_d[:, OFF_W1:OFF_W1 + 1600])
        w1_sb = {}
        for i, nm in enumerate(("ao", "ap", "bo", "bp")):
            t = pool1.tile([128, 800], F16, tag="w1" + nm)
            half = (i // 2) * 800               # a or b
            own = (i % 2) == 0
            for s in range(2):                  # s = direction of the feature block
                mcol = s if own else 2 + s
                nc.vector.tensor_scalar_mul(
                    t[:, s * 400:(s + 1) * 400],
                    w1full[:, half + s * 400:half + (s + 1) * 400],
                    masks_sb[:, mcol:mcol + 1])
            w1_sb[nm] = t

        # persistent xs buffers (memset once; garbage cols stay 0)
        xs_own = pool1.tile([128, L * 64], F32, tag="xs_own")
        xs_part = pool1.tile([128, L * 64], F32, tag="xs_part")
        nc.vector.memset(xs_own[:], 0.0)
        nc.vector.memset(xs_part[:], 0.0)
        # zero both gate-psum slots once (garbage lanes read later must be finite 0)
        for _ in range(2):
            z = gps.tile([128, 64], F32, tag="gps")
            nc.vector.memset(z[:], 0.0)

        def xs_matmul(lhsT_sb, ktiles, rhs_tiles, lay, into_own_only):
            """xs_{own,part}[t*64+pc] += (sum_k lhsT[k].T @ rhs[k])/WS (+bias for own)."""
            xov = xs_own[:].rearrange("p (t c) -> p t c", c=64)
            xpv = xs_part[:].rearrange("p (t c) -> p t c", c=64)
            for m in range(16):
                pc, rows, wc, q, k0 = mtile_info(m)
                pso = xps.tile([128, 256], F32, tag="xps")
                psp = None if into_own_only else xps.tile([128, 256], F32, tag="xps")
                for k in range(ktiles):
                    lw = lhsT_sb[:, k * 1600 + wc: k * 1600 + wc + rows]
                    ro, rp = rhs_tiles(k)
                    nc.tensor.matmul(pso[:rows, :], lw, ro, start=(k == 0), stop=(k == ktiles - 1))
                    if psp is not None:
                        nc.tensor.matmul(psp[:rows, :], lw, rp, start=(k == 0), stop=(k == ktiles - 1))
                biasap = bias_sb[:rows, lay * 16 + m: lay * 16 + m + 1]
                pv = pso[:rows, :].rearrange("p (t b) -> p t b", b=4)
                nc.vector.tensor_scalar(xov[:rows, :, pc:pc + 4], pv, 1.0 / WS, biasap,
                                        op0=ALU.mult, op1=ALU.add)
                if psp is not None:
                    pv2 = psp[:rows, :].rearrange("p (t b) -> p t b", b=4)
                    nc.vector.tensor_scalar(xpv[:rows, :, pc:pc + 4], pv2, 1.0 / WS, None,
                                            op0=ALU.mult)

        # ---- layer 0 xs ----
        x0v = x0T[:].rearrange("p (t c) -> p t c", c=16)
        wih0_sb = wihp.tile([128, 4 * 1600], IMG_DT, tag="wih", bufs=1)
        load_img(wih0_sb, IMG_WIH0, 6400)
        xs_matmul(wih0_sb, 4, lambda k: (x0v[:, :, k * 4:(k + 1) * 4], None), 0, True)

        def load_whh(l):
            t = whhp.tile([128, 4 * 1600], IMG_DT, tag="whh", name=f"whh{l}")
            load_img(t, IMG_WHH + l * 6400, 6400)
            return t

        def load_wihr(l):
            t = wihp.tile([128, 8 * 1600], IMG_DT, tag="wihr", name=f"wihr{l}")
            load_img(t, IMG_WIHR + (l - 1) * 12800, 12800)
            return t

        whh_l = load_whh(0)
        wihr_next = load_wihr(1) if n_layers > 1 else None
        xin_f = None  # final exchange tile
        for l in range(n_layers):
            if l > 0:
                whh_l = load_whh(l)
            xout = xop.tile([128, 16 * L], F16, tag="xout")
            xov_ = xout[:].rearrange("p (t c) -> p t c", c=16)
            c_t = sp.tile([128, 16], F32, tag="c")
            nc.vector.memset(c_t[:], 0.0)
            for t in range(L):
                g = gp.tile([128, 64], F32, tag="g")
                if t == 0:
                    src = xs_own[:, 0:64]
                    nc.scalar.activation(g[:, 0:48], src[:, 0:48], AF.Sigmoid)
                    nc.scalar.activation(g[:, 48:64], src[:, 48:64], AF.Tanh)
                else:
                    ps = gps.tile([128, 64], F32, tag="gps")
                    for m in range(16):
                        pc, rows, wc, q, k0 = mtile_info(m)
                        for k in range(4):
                            nc.tensor.matmul(
                                ps[:rows, pc:pc + 4],
                                whh_l[:, k * 1600 + wc: k * 1600 + wc + rows],
                                xov_[:, t - 1, k * 4:(k + 1) * 4],
                                start=(k == 0), stop=(k == 3))
                    nc.vector.scalar_tensor_tensor(g[:], ps[:], 1.0 / WS,
                                                   xs_own[:, t * 64:(t + 1) * 64],
                                                   op0=ALU.mult, op1=ALU.add)
                    nc.scalar.activation(g[:, 0:48], g[:, 0:48], AF.Sigmoid)
                    nc.scalar.activation(g[:, 48:64], g[:, 48:64], AF.Tanh)
                tmp = sp.tile([128, 16], F32, tag="tmp")
                nc.vector.tensor_mul(tmp[:], g[:, 0:16], g[:, 48:64])      # i*tanh(g)
                nc.vector.tensor_mul(c_t[:], g[:, 16:32], c_t[:])          # f*c
                nc.vector.tensor_add(c_t[:], c_t[:], tmp[:])
                thc = sp.tile([128, 16], F32, tag="thc")
                nc.scalar.activation(thc[:], c_t[:], AF.Tanh)
                nc.vector.tensor_mul(xov_[:, t, :], g[:, 32:48], thc[:])   # h (fp16 out)

            # ---- exchange: AllGather over the {fwd,bwd} pair ----
            nc.sync.dma_start(exin[l].ap(), xout[:])
            if no_collective:
                for r in range(2):
                    nc.sync.dma_start(exout[l].ap()[r], exin[l].ap())
            else:
                nc.gpsimd.collective_compute(
                    "AllGather", ALU.bypass, ins=[exin[l].ap().opt()],
                    outs=[exout[l].ap().opt()], replica_groups=groups)
            xin = xip.tile([128, 2 * 16 * L], F16, tag="xin")
            for s in range(2):
                nc.sync.dma_start(xin[:, s * 1024:(s + 1) * 1024], exout[l].ap()[s])
            if l < n_layers - 1:
                xin_o = xip.tile([128, 2048], F16, tag="xin_o", bufs=1)
                xin_p = xip.tile([128, 2048], F16, tag="xin_p", bufs=1)
                for s in range(2):
                    sl = slice(s * 1024, (s + 1) * 1024)
                    nc.vector.tensor_scalar_mul(xin_o[:, sl], xin[:, sl], masks_sb[:, s:s + 1])
                    nc.vector.tensor_scalar_mul(xin_p[:, sl], xin[:, sl], masks_sb[:, 2 + s:3 + s])
                xiov = xin_o[:].rearrange("p (s t c) -> p s t c", s=2, c=16)
                xipv = xin_p[:].rearrange("p (s t c) -> p s t c", s=2, c=16)
                wihr_cur, wihr_next = wihr_next, (load_wihr(l + 2) if l + 2 < n_layers else None)
                xs_matmul(wihr_cur, 8,
                          lambda k: (xiov[:, k // 4, :, (k % 4) * 4:(k % 4) * 4 + 4],
                                     xipv[:, k // 4, :, (k % 4) * 4:(k % 4) * 4 + 4]),
                          l + 1, False)
                # fold xs_part (read time-reversed) into xs_own once per layer
                # so the recurrence needs a single add per step
                xoc = xs_own[:].rearrange("p (t c) -> p t c", c=64)
                xpc = xs_part[:].rearrange("p (t c) -> p t c", c=64)
                nc.vector.tensor_add(xoc, xoc, xpc[:, ::-1, :])
            else:
                xin_f = xin

        # ---------------- phases ----------------
        if dbg == "ag":
            # dump agout[r][:, 0:16] for r=0..3 (as f32) for bit-level AG check
            dago = php.tile([128, 64], F8, tag="dago")
            for r in range(4):
                nc.sync.dma_start(dago[:, r * 16:(r + 1) * 16], agout.ap()[r][:, 0:16])
            dagf = php.tile([128, 64], F32, tag="dagf")
            nc.vector.tensor_copy(dagf[:], dago[:])
            nc.sync.dma_start(out_d[:, :], dagf[:])
        elif dbg == "xs":
            dxs = php.tile([128, 64], F32, tag="dxs")
            nc.vector.tensor_copy(dxs[:], xs_own[:, 0:64])
            nc.sync.dma_start(out_d[:, :], dxs[:])
        elif no_head:
            out_sb0 = php.tile([128, 64], F32, tag="osb0")
            nc.vector.tensor_copy(out_sb0[:, 0:64], xin_f[:, 0:64])
            nc.sync.dma_start(out_d[:, :], out_sb0[:])
        else:
            # P = poly(x) per slot, then masked own/part
            # quintic via 5 fused ops: P = ((((w5 f + w4) f + w3) f + w2) f + w1) f
            P = php.tile([128, 2048], F32, tag="P")
            x_f = xin_f[:]
            nc.vector.tensor_scalar(P[:], x_f, consts_sb[:, 4:5], consts_sb[:, 3:4],
                                    op0=ALU.mult, op1=ALU.add)
            t1 = php.tile([128, 2048], F32, tag="t1")
            nc.vector.scalar_tensor_tensor(t1[:], P[:], 0.0, x_f,
                                           op0=ALU.add, op1=ALU.mult)
            nc.vector.scalar_tensor_tensor(P[:], t1[:], consts_sb[:, 2:3], x_f,
                                           op0=ALU.add, op1=ALU.mult)
            nc.vector.scalar_tensor_tensor(t1[:], P[:], consts_sb[:, 1:2], x_f,
                                           op0=ALU.add, op1=ALU.mult)
            nc.vector.scalar_tensor_tensor(P[:], t1[:], consts_sb[:, 0:1], x_f,
                                           op0=ALU.add, op1=ALU.mult)
            Po = php.tile([128, 2048], F16, tag="Po")
            Pp = php.tile([128, 2048], F16, tag="Pp")
            for s in range(2):
                sl = slice(s * 1024, (s + 1) * 1024)
                nc.vector.tensor_scalar_mul(Po[:, sl], P[:, sl], masks_sb[:, s:s + 1])
                nc.vector.tensor_scalar_mul(Pp[:, sl], P[:, sl], masks_sb[:, 2 + s:3 + s])
            Pov = Po[:].rearrange("p (s t c) -> p s t c", s=2, c=16)
            Ppv = Pp[:].rearrange("p (s t c) -> p s t c", s=2, c=16)

            def halfmat(wown, wpart):
                pso = pps.tile([128, 256], F32, tag="pps")
                psp = pps.tile([128, 256], F32, tag="pps")
                for k in range(8):
                    s, j = k // 4, k % 4
                    nc.tensor.matmul(pso[:100, :], wown[:, k * 100:(k + 1) * 100],
                                     Pov[:, s, :, j * 4:j * 4 + 4], start=(k == 0), stop=(k == 7))
                    nc.tensor.matmul(psp[:100, :], wpart[:, k * 100:(k + 1) * 100],
                                     Ppv[:, s, :, j * 4:j * 4 + 4], start=(k == 0), stop=(k == 7))
                return pso, psp

            def rev_add(dst, pso, psp):
                # dst = (pso + reverse_t(psp)) / WS
                pv = psp[:100, :].rearrange("p (t b) -> p t b", b=4)
                dv = dst[:100, :].rearrange("p (t b) -> p t b", b=4)
                nc.vector.tensor_scalar(dst[:100, :], pso[:100, :], 1.0 / WS, None,
                                        op0=ALU.mult)
                nc.vector.scalar_tensor_tensor(dv, pv[:, ::-1, :], 1.0 / WS, dv,
                                               op0=ALU.mult, op1=ALU.add)

            A_sb = php.tile([128, 256], F32, tag="A")
            C_sb = php.tile([128, 256], F32, tag="C")
            pso, psp = halfmat(w1_sb["ao"], w1_sb["ap"])
            rev_add(A_sb, pso, psp)
            nc.vector.tensor_scalar(A_sb[:100, :], A_sb[:100, :], consts_sb[:100, 5:6], None,
                                    op0=ALU.add)  # + b1
            pso, psp = halfmat(w1_sb["bo"], w1_sb["bp"])
            rev_add(C_sb, pso, psp)

            out_sb = php.tile([128, 64], F32, tag="osb")
            Cv = C_sb[:100, :].rearrange("p (t b) -> p t b", b=4)
            ps4 = pps.tile([128, 64], F32, tag="ps4", bufs=1)
            for a in range(32):
                hm = gp.tile([128, 256], F16, tag="hm")
                for b in range(4):
                    nc.vector.tensor_scalar(hm[:100, b * 64:(b + 1) * 64], Cv[:, :, b],
                                            A_sb[:100, a * 4 + b:a * 4 + b + 1], 0.0,
                                            op0=ALU.add, op1=ALU.max)
                for ch in range(2):
                    # out[bc, 0] = sum_k hm[k, ch*128+bc] * W2[k]
                    nc.tensor.matmul(ps4[:, a * 2 + ch:a * 2 + ch + 1],
                                     hm[:100, ch * 128:(ch + 1) * 128],
                                     scal16[:100, 84 + 7:84 + 8])
            nc.vector.tensor_scalar(out_sb[:, :], ps4[:, :], consts_sb[0:128, 6:7], None,
                                    op0=ALU.add)  # + b2
            nc.sync.dma_start(out_d[:, :], out_sb[:])
    nc.compile()
    return nc


_CACHE = {}


def _get_program():
    if "nc" not in _CACHE:
        _CACHE["nc"] = build_program()
    return _CACHE["nc"]


def _prep_core_inputs(c, words, pos, w_emb, t_emb, Wih0, Wih_rest, Whh, bih, bhh,
                      ws, mlp_W1, mlp_b1, mlp_W2, mlp_b2):
    d, g = c // 4, c % 4
    bs = slice(4 * g, 4 * g + 4)
    # x0T: (128, 1024) f16, col = t*16 + j*4 + b  (slot-order time)
    X = np.concatenate([w_emb[words[bs]], t_emb[pos[bs]]], axis=-1)  # (4,64,400)
    if d == 1:
        X = X[:, ::-1]
    Xp = np.concatenate([X, np.zeros((4, 64, 112), X.dtype)], -1)    # pad 512
    x0T = Xp.reshape(4, 64, 4, 128).transpose(3, 1, 2, 0).reshape(128, 1024)

    # fp8 weight image for this direction, then this core's quarter
    img = np.concatenate(
        [_kmaj(_prep_lhsT(Whh[l, d], 1)) for l in range(NL)]
        + [_kmaj(_prep_lhsT(Wih0[d], 1))]
        + [_kmaj(_prep_lhsT(Wih_rest[l - 1, d], 2)) for l in range(1, NL)],
        axis=1)
    assert img.shape == (128, IMG_COLS)
    wq = _img_pack(img[:, g * QCOLS:(g + 1) * QCOLS])

    bias = np.stack([_prep_bias(bih[l, d] + bhh[l, d]) for l in range(NL)])  # (5,16,128)
    bias = bias.reshape(NL * 16, 128).T                                      # (128, 80)
    masks = np.zeros((4, 128), np.float32)
    masks[0] = float(d == 0); masks[1] = float(d == 1)
    masks[2] = float(d == 1); masks[3] = float(d == 0)
    consts = np.zeros((8, 128), np.float32)
    for i in range(5):
        consts[i] = ws[i]
    consts[5, :100] = mlp_b1
    consts[6] = mlp_b2[0]
    consts[7, :100] = mlp_W2[0]
    w1a = _prep_w1(mlp_W1[:, :800])
    w1b = _prep_w1(mlp_W1[:, 800:])
    blob = np.concatenate([
        x0T.astype(np.float16),
        bias.astype(np.float16),
        masks.T.astype(np.float16),
        consts.T.astype(np.float16),
        _kmaj(w1a).astype(np.float16),
        _kmaj(w1b).astype(np.float16),
        wq,
    ], axis=1)
    assert blob.shape == (128, NBLOB), blob.shape
    return {"blob": np.ascontiguousarray(blob)}


def _postprocess(results):
    out = np.zeros((L, B, L, 1), np.float32)
    ar = np.arange(32)
    ac = np.arange(L)
    for c in range(NCORES):
        d, g = c // 4, c % 4
        arr = results[c]["out"]                     # (128, 64): [p, a*2+ch]
        # value at [p, a*2+ch] belongs to (a, bl=ch*2+p//64, c=p%64)
        ch = arr.reshape(2, 64, 32, 2).transpose(2, 3, 0, 1).reshape(32, 4, 64)
        for bl in range(4):
            if d == 0:
                out[ar[:, None], 4 * g + bl, ac[None, :], 0] = ch[:, bl, :]
            else:
                out[(63 - ar)[:, None], 4 * g + bl, (63 - ac)[None, :], 0] = ch[:, bl, :]
    return out


def prepare(words_idx_tensor, pos_idx_tensor, max_length, w_emb, t_emb, Wih0, Wih_rest,
            Whh, bih, bhh, w1, w2, w3, w4, w5, mlp_W1, mlp_b1, mlp_W2, mlp_b2):
    words = np.asarray(words_idx_tensor)[:, :int(max_length)].astype(np.int64)
    pos = np.asarray(pos_idx_tensor)[:, :int(max_length)].astype(np.int64)
    assert words.shape == (B, L)
    args = tuple(np.asarray(x, np.float32) for x in
                 (w_emb, t_emb, Wih0, Wih_rest, Whh, bih, bhh))
    ws = [float(np.asarray(w).reshape(-1)[0]) for w in (w1, w2, w3, w4, w5)]
    mW1, mb1, mW2, mb2 = (np.asarray(mlp_W1, np.float32), np.asarray(mlp_b1, np.float32),
                          np.asarray(mlp_W2, np.float32), np.asarray(mlp_b2, np.float32))
    in_maps = [_prep_core_inputs(c, words, pos, *args, ws, mW1, mb1, mW2, mb2)
               for c in range(NCORES)]
    nc = _get_program()
    return nc, in_maps, _postprocess


def kernel(words_idx_tensor, pos_idx_tensor, max_length, w_emb, t_emb, Wih0, Wih_rest,
           Whh, bih, bhh, w1, w2, w3, w4, w5, mlp_W1, mlp_b1, mlp_W2, mlp_b2,
           _stats=None, _trace=False):
    nc, in_maps, post = prepare(
        words_idx_tensor, pos_idx_tensor, max_length, w_emb, t_emb, Wih0, Wih_rest,
        Whh, bih, bhh, w1, w2, w3, w4, w5, mlp_W1, mlp_b1, mlp_W2, mlp_b2)
    res = run_bass_kernel_spmd(nc, in_maps, list(range(NCORES)), trace=_trace)
    if _stats is not None:
        _stats["exec_time_ns"] = res.exec_time_ns
        _stats["mean_exec_time_ns"] = res.mean_exec_time_ns
        _stats["profile_json"] = res.profile_json
    return _postprocess(res.results)
